# revision 27
# baseline (speedup 1.0000x reference)
"""Trainium2 Bass kernel for AttentionWithGeGLU pooling.

Math (per batch row b):
  q[s]   = sum_d x[b,s,d]^2
  rs[s]  = (q/D + eps)^-1/2          (1 Newton step from y0 = 1.5 - 0.5 v)
  t[s]   = sum_d x[b,s,d] * (ln_w*att_w)[d]
  score  = rs * t                    (att_b dropped: softmax shift-invariant)
  e      = exp(score)
  praw[b,d] = sum_s (e[s]*rs[s]) * x[b,s,d]
  pooled = praw / sum_s e            (host; e shipped out raw)
  h      = pooled @ (ln_w[:,None]*geglu_w) + geglu_b;  out = val * gelu(gate)

Two NEFF launches (collective latency ~70us makes a fused NEFF slower).

Pool NEFF engine split per x tile [128,1024]: ACT computes q via
Square+accum_out (DVE takes QDVE of them for balance), DVE computes t via
affine_mul_reduce, GpSimd runs the softmax smalls, PE accumulates value
matmuls in bf16, ACT evacuates PSUM.  The exp/c/value-matmul/evac chain of
each half-batch chunk is EMITTED one chunk late: engines execute their
queues in order, so an instruction whose cross-engine deps aren't met yet
(exp waits on GpSimd, evac waits on PE) head-of-line-blocks everything
behind it.  Deferring the emission point keeps ACT/DVE streaming.  The
final chunk's smalls run on DVE (idle by then) to shorten the tail.
"""

import os
import numpy as np

B, S, D, OUT = 32, 2048, 1024, 4096
EPS = 1e-6
NCORES = 8
NB = B // NCORES          # batches per core
COLS = OUT // NCORES      # val columns per core
P = 128
NT = S // P               # seq tiles per batch
NC = 2                    # softmax chunks per batch
CT = NT // NC             # tiles per chunk
QDVE = 4                  # q-tiles offloaded ACT->DVE (balance), not in last chunk
STRIPE0 = 2               # first tiles striped across 4 DMA queues

_cache = {}


def _build_nc_pool():
    import concourse.bacc as bacc
    import concourse.mybir as mybir
    import concourse.tile as tile
    from contextlib import ExitStack

    f32 = mybir.dt.float32
    bf16 = mybir.dt.bfloat16
    AF = mybir.ActivationFunctionType
    OP = mybir.AluOpType

    nc = bacc.Bacc(
        "TRN2",
        target_bir_lowering=False,
        debug=False,
        enable_asserts=False,
        num_devices=NCORES,
    )

    x_d = nc.dram_tensor("x", [NB, S, D], bf16, kind="ExternalInput").ap()
    a_d = nc.dram_tensor("a", [1, D], bf16, kind="ExternalInput").ap()
    praw_d = nc.dram_tensor("praw", [NB, D], f32, kind="ExternalOutput").ap()
    e_d = nc.dram_tensor("e", [NB, P, NT], f32, kind="ExternalOutput").ap()

    ntiles = NB * NT
    qdve_set = set()
    if QDVE:
        step = (ntiles - CT) // QDVE
        qdve_set = {step // 2 + i * step for i in range(QDVE)}

    # chunk table: (batch, start tile, len); the last batch tapers to 2-tile
    # chunks so the serial tail is short
    chunks = []
    for b in range(NB):
        if b < NB - 1:
            for c in range(NC):
                chunks.append((b, c * CT, CT))
        else:
            chunks.append((b, 0, CT))
            chunks.append((b, CT, 4))
            chunks.append((b, CT + 4, 2))
            chunks.append((b, CT + 6, 2))
    NCH = len(chunks)

    with tile.TileContext(nc) as tc, ExitStack() as ctx:
        singles = ctx.enter_context(tc.tile_pool(name="singles", bufs=1))
        xpool = ctx.enter_context(tc.tile_pool(name="xp", bufs=28))
        scr_a = ctx.enter_context(tc.tile_pool(name="scra", bufs=3))
        scr_v = ctx.enter_context(tc.tile_pool(name="scrv", bufs=3))
        small = ctx.enter_context(tc.tile_pool(name="small", bufs=4))
        psum_pool = ctx.enter_context(
            tc.tile_pool(name="pspool", bufs=1, space="PSUM")
        )

        # tile 0's DMA strips are emitted first (descriptor generation on the
        # sync engine serializes at ~600ns per dma_start; the first compute
        # waits on tile 0); a_bc lands during tile 0's squares
        first_xt = xpool.tile([P, D], bf16, tag="x", name="xt0")
        for st in range(4):
            nc.sync.dma_start(out=first_xt[st * 32:(st + 1) * 32, :],
                              in_=x_d[0, st * 32:(st + 1) * 32, :])
        a_bc = singles.tile([P, D], bf16)
        for st in range(4):
            nc.sync.dma_start(out=a_bc[st * 32:(st + 1) * 32, :],
                              in_=a_d.to_broadcast([32, D]))

        pp_of = {}          # batch -> psum tile
        e_of = {}           # batch -> e_all tile
        chunk_info = {}     # g -> dict(tiles, y1, sc, ...)

        def emit_block(g):
            """exp, c, value matmuls (+ e DMA at batch end) for chunk g."""
            b, cs, cl = chunks[g]
            info = chunk_info.pop(g)
            gv = info["g_eng"]
            if b not in e_of:
                e_of[b] = small.tile([P, NT], f32, tag="e", name=f"eall{b}")
            e_all = e_of[b]
            esl = e_all[:, cs:cs + cl]
            nc.scalar.activation(out=esl, in_=info["sc"], func=AF.Exp)
            c_all = small.tile([P, cl], bf16, tag="c", name=f"call{g}")
            gv.tensor_mul(c_all, esl, info["y1"])
            if cs + cl == NT:
                nc.sync.dma_start(out=e_d[b], in_=e_all)
                e_of.pop(b)
            # batches 0-2 accumulate at partition offsets {0,32,64} of one
            # PSUM tile (single evac copy); batch 3 gets its own tile
            if b < 3:
                if "p65" not in pp_of:
                    pp_of["p65"] = psum_pool.tile([65, D], f32, tag="acc65",
                                                  name="pp65")
                pp, boff = pp_of["p65"], 32 * b
            else:
                if "p3" not in pp_of:
                    pp_of["p3"] = psum_pool.tile([1, D], f32, tag="acc3",
                                                 name="pp3")
                pp, boff = pp_of["p3"], 0
            for j in range(cl):
                for h in range(2):
                    nc.tensor.matmul(
                        pp[boff:boff + 1, h * 512:(h + 1) * 512],
                        lhsT=c_all[:, j:j + 1],
                        rhs=info["tiles"][j][:, h * 512:(h + 1) * 512],
                        start=(cs == 0 and j == 0),
                        stop=(cs + cl == NT and j == cl - 1))

        def emit_evac65():
            pp = pp_of.pop("p65")
            pr_sb = small.tile([65, D], f32, tag="pr65", name="pr65")
            nc.scalar.copy(pr_sb, pp)
            for b in range(3):
                nc.sync.dma_start(out=praw_d[b:b + 1, :],
                                  in_=pr_sb[32 * b:32 * b + 1, :])

        def emit_evac3():
            pp = pp_of.pop("p3")
            pr_sb = small.tile([1, D], f32, tag="pr")
            nc.scalar.copy(pr_sb, pp)
            nc.sync.dma_start(out=praw_d[3:4, :], in_=pr_sb)

        for g in range(NCH):
            b, cs, cl = chunks[g]
            last = (g == NCH - 1)
            # ---- step 1: stream this chunk's tiles; q on ACT, t on DVE ----
            x_tiles = []
            q_all = small.tile([P, cl], f32, tag="q", name=f"q{g}")
            t_all = small.tile([P, cl], f32, tag="t", name=f"t{g}")
            for j in range(cl):
                jj = cs + j
                gt = b * NT + jj
                if gt == 0:
                    xt = first_xt
                else:
                    xt = xpool.tile([P, D], bf16, tag="x")
                    if gt < STRIPE0:
                        for st in range(4):
                            nc.sync.dma_start(
                                out=xt[st * 32:(st + 1) * 32, :],
                                in_=x_d[b, jj * P + st * 32:
                                        jj * P + (st + 1) * 32, :])
                    else:
                        nc.sync.dma_start(
                            out=xt, in_=x_d[b, jj * P:(jj + 1) * P, :])
                x_tiles.append(xt)
                if gt in qdve_set:
                    sq = scr_v.tile([P, D], bf16, tag="sqv")
                    nc.vector.affine_mul_reduce(
                        out=sq, accum_out=q_all[:, j:j + 1],
                        in0=xt, in1=xt, scale=1.0, bias=0.0)
                else:
                    sq = scr_a.tile([P, D], bf16, tag="sqa")
                    nc.scalar.activation(out=sq, in_=xt, func=AF.Square,
                                         accum_out=q_all[:, j:j + 1])
                tp = scr_v.tile([P, D], bf16, tag="tp")
                nc.vector.affine_mul_reduce(
                    out=tp, accum_out=t_all[:, j:j + 1],
                    in0=xt, in1=a_bc, scale=1.0, bias=0.0)

            # ---- step 2: deferred work whose deps are met by now ----
            if g >= 1:
                emit_block(g - 1)

            # ---- step 3: softmax smalls (GpSimd; DVE for the last two
            # chunks, which are off the GpSimd pipeline by then) ----
            gv = nc.vector if g >= NCH - 2 else nc.gpsimd
            v = small.tile([P, cl], f32, tag="v", name=f"v{g}")
            gv.tensor_scalar(out=v, in0=q_all, scalar1=1.0 / D,
                             scalar2=EPS, op0=OP.mult, op1=OP.add)
            y = small.tile([P, cl], f32, tag="y", name=f"y{g}")
            gv.tensor_scalar(out=y, in0=v, scalar1=-0.5, scalar2=1.5,
                             op0=OP.mult, op1=OP.add)
            u = small.tile([P, cl], f32, tag="u", name=f"u{g}")
            gv.tensor_mul(u, y, y)
            gv.tensor_mul(u, u, v)
            gv.tensor_scalar(out=u, in0=u, scalar1=-0.5, scalar2=1.5,
                             op0=OP.mult, op1=OP.add)
            y1 = small.tile([P, cl], f32, tag="y1", name=f"y1_{g}")
            gv.tensor_mul(y1, y, u)
            sc = small.tile([P, cl], f32, tag="sc", name=f"sc{g}")
            gv.tensor_mul(sc, t_all, y1)
            chunk_info[g] = {"tiles": x_tiles, "y1": y1, "sc": sc,
                             "g_eng": gv}

        # ---- tail: final chunk's block + evacs (the 65-row copy runs on
        # ACT while DVE computes the last chunk's smalls) ----
        emit_evac65()
        emit_block(NCH - 1)
        emit_evac3()

    nc.compile()
    return nc


def _build_nc_geglu():
    import concourse.bacc as bacc
    import concourse.mybir as mybir
    import concourse.tile as tile
    from contextlib import ExitStack

    f32 = mybir.dt.float32
    bf16 = mybir.dt.bfloat16
    AF = mybir.ActivationFunctionType

    nc = bacc.Bacc(
        "TRN2",
        target_bir_lowering=False,
        debug=False,
        enable_asserts=False,
        num_devices=NCORES,
    )

    pT_d = nc.dram_tensor("pT", [P, 8, B], bf16, kind="ExternalInput").ap()
    w_d = nc.dram_tensor("w", [8, P, 2 * COLS], bf16, kind="ExternalInput").ap()
    bias_d = nc.dram_tensor("bias", [1, 2 * COLS], f32, kind="ExternalInput").ap()
    out_d = nc.dram_tensor("out", [B, COLS], f32, kind="ExternalOutput").ap()

    with tile.TileContext(nc) as tc, ExitStack() as ctx:
        singles = ctx.enter_context(tc.tile_pool(name="singles", bufs=1))
        tailp = ctx.enter_context(tc.tile_pool(name="tail", bufs=2))
        psum_pool = ctx.enter_context(
            tc.tile_pool(name="pspool", bufs=1, space="PSUM")
        )

        pT_sb = singles.tile([P, 8, B], bf16)
        nc.sync.dma_start(out=pT_sb, in_=pT_d)
        # per-chunk DMAs (2 partition-strips each -> 16 queues) so matmul k
        # starts as soon as chunk k lands
        w_sb = singles.tile([P, 8, 2 * COLS], bf16)
        for k in range(8):
            nc.sync.dma_start(out=w_sb[:, k], in_=w_d[k])
        bias_bc = singles.tile([B, 2 * COLS], f32)
        nc.sync.dma_start(out=bias_bc, in_=bias_d.to_broadcast([B, 2 * COLS]))

        hps = psum_pool.tile([B, 2 * COLS], f32, tag="acc")
        for k in range(8):
            for h in range(2):
                nc.tensor.matmul(
                    hps[:, h * COLS:(h + 1) * COLS],
                    lhsT=pT_sb[:, k, :],
                    rhs=w_sb[:, k, h * COLS:(h + 1) * COLS],
                    start=(k == 0), stop=(k == 7))
        # gate half first so ACT's gelu overlaps the value-half add on DVE
        hg = tailp.tile([B, COLS], f32, tag="hg")
        nc.vector.tensor_add(hg, hps[:, COLS:2 * COLS], bias_bc[:, COLS:2 * COLS])
        gg = tailp.tile([B, COLS], f32, tag="gg")
        nc.scalar.activation(out=gg, in_=hg, func=AF.Gelu)
        hv = tailp.tile([B, COLS], f32, tag="hv")
        nc.vector.tensor_add(hv, hps[:, 0:COLS], bias_bc[:, 0:COLS])
        outt = tailp.tile([B, COLS], f32, tag="outt")
        nc.vector.tensor_mul(outt, hv, gg)
        nc.sync.dma_start(out=out_d, in_=outt)

    nc.compile()
    return nc


def _pool_in_maps(x, ln_w, att_w):
    import ml_dtypes
    bf = ml_dtypes.bfloat16
    a = (ln_w * att_w[:, 0]).astype(bf).reshape(1, D)
    xc = np.ascontiguousarray(x.astype(bf))
    return [
        {"x": xc[r * NB:(r + 1) * NB], "a": a}
        for r in range(NCORES)
    ]


def _geglu_in_maps(pooled_full, ln_w, geglu_w, geglu_b):
    import ml_dtypes
    bf = ml_dtypes.bfloat16
    pT = np.ascontiguousarray(
        pooled_full.T.astype(bf).reshape(8, P, B).transpose(1, 0, 2))
    Wf = ln_w[:, None] * geglu_w
    maps = []
    for r in range(NCORES):
        vs = slice(r * COLS, (r + 1) * COLS)
        gs = slice(OUT + r * COLS, OUT + (r + 1) * COLS)
        wr = np.ascontiguousarray(
            np.concatenate([Wf[:, vs], Wf[:, gs]], axis=1)
            .astype(bf).reshape(8, P, 2 * COLS))
        br = np.ascontiguousarray(
            np.concatenate([geglu_b[vs], geglu_b[gs]])
        ).reshape(1, 2 * COLS).astype(np.float32)
        maps.append({"pT": pT, "w": wr, "bias": br})
    return maps


LAST_RESULTS = None


def kernel(x, ln_w, att_w, att_b, geglu_w, geglu_b):
    global LAST_RESULTS
    from concourse.bass_utils import run_bass_kernel_spmd

    x = np.asarray(x, dtype=np.float32)
    ln_w = np.asarray(ln_w, dtype=np.float32)
    att_w = np.asarray(att_w, dtype=np.float32)
    geglu_w = np.asarray(geglu_w, dtype=np.float32)
    geglu_b = np.asarray(geglu_b, dtype=np.float32)
    # att_b is mathematically irrelevant (softmax shift-invariance)

    if "A" not in _cache:
        _cache["A"] = _build_nc_pool()
    if "B" not in _cache:
        _cache["B"] = _build_nc_geglu()

    trace = os.environ.get("KERNEL_TRACE", "0") == "1"

    res_a = run_bass_kernel_spmd(
        _cache["A"], _pool_in_maps(x, ln_w, att_w),
        core_ids=list(range(NCORES)), trace=trace,
    )
    praw = np.concatenate(
        [res_a.results[r]["praw"] for r in range(NCORES)], axis=0
    ).astype(np.float64)
    esum = np.stack(
        [res_a.results[r]["e"].astype(np.float64).sum(axis=(1, 2))
         for r in range(NCORES)]
    ).reshape(B)
    pooled_full = (praw / esum[:, None]).astype(np.float32)

    res_b = run_bass_kernel_spmd(
        _cache["B"], _geglu_in_maps(pooled_full, ln_w, geglu_w, geglu_b),
        core_ids=list(range(NCORES)), trace=trace,
    )
    LAST_RESULTS = (res_a, res_b)
    out = np.concatenate(
        [res_b.results[r]["out"] for r in range(NCORES)], axis=1
    )
    return out.astype(np.float32)
